# revision 23
# baseline (speedup 1.0000x reference)
"""Trainium2 Bass kernel for nn_MessageLayer (GNN message passing).

Strategy
--------
* Host: sort edges by self_fea_idx; pack sorted edges into "windows" of
  T_WIN*128 = 1024 edge slots whose node span is < 128 (so per-window
  segment reductions fit one 128-wide one-hot matmul).  Windows are
  distributed contiguously over the 8 cores; every core runs the exact
  same program (SPMD) on its own slice.
* Device (per core, per window):
    - edge MLP (5 layers, 128->128) feature-major, bf16 matmuls,
      fused bias+LeakyReLU on the scalar engine (PSUM->SBUF).
    - gate/msg layer-1 (256->256) feature-major.
    - layer-2 matmuls emit EDGE-major tiles directly by using the hidden
      activations as the stationary operand (no transposes).
    - g = exp(graw) * w^p  (softmax max-shift eliminated algebraically:
      a per-segment shift cancels in the ratio).
    - segment sums of [g*m | g] via one-hot matmuls accumulated in PSUM.
* Host post: accumulate per-window partials into (N, H, 64) sums and
  (N, H) softmax denominators, apply msg bias fold, normalization, head
  mean, residual; gates = g / (Z[self_idx] + 1e-10).

The kernel returns (out, gates) exactly like the reference:
  out: (N, 64) float32, gates: (H, M, 1) float32.
"""

import os
import sys
from contextlib import ExitStack

import numpy as np

for _p in ("/opt/trn_rl_repo",):
    if _p not in sys.path:
        sys.path.insert(0, _p)

import ml_dtypes  # noqa: E402
import concourse.bass as bass  # noqa: E402
import concourse.bacc as bacc  # noqa: E402
import concourse.mybir as mybir  # noqa: E402
import concourse.tile as tile  # noqa: E402

AFT = mybir.ActivationFunctionType
ALU = mybir.AluOpType
F32 = mybir.dt.float32
BF16 = mybir.dt.bfloat16

P = 128
H = 3
ELEM = 64
EDGE = 128
HID = 256
T_WIN = 8                 # 128-edge subtiles per window
SLOTS_WIN = T_WIN * P     # 1024 edge slots per window
SEG_COLS = H * ELEM + H   # 195: [gm heads | g heads]
N_CORES = 8

# matmul/activation dtype for the MLP path (fp32 for max accuracy, bf16 fast)
_DT_ENV = os.environ.get("GNN_KERNEL_DT", "bf16")
DT = BF16 if _DT_ENV == "bf16" else F32
NP_DT = ml_dtypes.bfloat16 if _DT_ENV == "bf16" else np.float32

LAST_RESULTS = None  # BassKernelResults of the most recent device run
_PROGRAM_CACHE = {}


# --------------------------------------------------------------------------
# device program
# --------------------------------------------------------------------------

def build_program(W, dt=DT, act_fn=AFT.Prelu):
    """Build the SPMD per-core Bass program for W windows per core."""
    n_sub = W * T_WIN
    n_slots = n_sub * P
    nc = bacc.Bacc("TRN2", target_bir_lowering=False, debug=False)

    xsn = nc.dram_tensor("xsn", [P, n_slots], dt, kind="ExternalInput")
    edg = nc.dram_tensor("edg", [P, n_slots], dt, kind="ExternalInput")
    idxr = nc.dram_tensor("idxr", [P, n_sub], F32, kind="ExternalInput")
    wp = nc.dram_tensor("wp", [P, 3 * n_sub], F32, kind="ExternalInput")
    ew = nc.dram_tensor("ew", [P, 5 * P], dt, kind="ExternalInput")
    eb = nc.dram_tensor("eb", [P, 5], F32, kind="ExternalInput")
    w1 = nc.dram_tensor("w1", [P, 24 * P], dt, kind="ExternalInput")
    b1 = nc.dram_tensor("b1", [P, 12], F32, kind="ExternalInput")
    g2w = nc.dram_tensor("g2w", [P, 6], dt, kind="ExternalInput")
    m2w = nc.dram_tensor("m2w", [P, 6 * ELEM], dt, kind="ExternalInput")
    gout = nc.dram_tensor("gout", [P, 3 * n_sub], F32, kind="ExternalOutput")
    sout = nc.dram_tensor("sout", [P, W * SEG_COLS], F32, kind="ExternalOutput")

    with ExitStack() as ctx:
        is16 = dt == BF16
        tc = ctx.enter_context(tile.TileContext(nc))
        const = ctx.enter_context(tc.tile_pool(name="const", bufs=1))
        io = ctx.enter_context(tc.tile_pool(name="io", bufs=3 if is16 else 2))
        actp = ctx.enter_context(tc.tile_pool(name="act", bufs=2 if is16 else 1))
        small = ctx.enter_context(tc.tile_pool(name="small", bufs=4))
        ps_big = ctx.enter_context(tc.tile_pool(name="ps_big", bufs=2, space="PSUM"))
        ps_gp = ctx.enter_context(tc.tile_pool(name="ps_g", bufs=1, space="PSUM"))
        ps_mp = ctx.enter_context(tc.tile_pool(name="ps_m", bufs=2, space="PSUM"))
        ps_wp = ctx.enter_context(tc.tile_pool(name="ps_w", bufs=1, space="PSUM"))

        ew_t = const.tile([P, 5 * P], dt)
        nc.sync.dma_start(ew_t[:], ew[:])
        eb_t = const.tile([P, 5], F32)
        nc.sync.dma_start(eb_t[:], eb[:])
        w1_t = const.tile([P, 24 * P], dt)
        nc.sync.dma_start(w1_t[:], w1[:])
        b1_t = const.tile([P, 12], F32)
        nc.sync.dma_start(b1_t[:], b1[:])
        g2w_t = const.tile([P, 6], dt)
        nc.sync.dma_start(g2w_t[:], g2w[:])
        m2w_t = const.tile([P, 6 * ELEM], dt)
        nc.sync.dma_start(m2w_t[:], m2w[:])
        idxr_t = const.tile([P, n_sub], F32)
        nc.sync.dma_start(idxr_t[:], idxr[:])
        wp_t = const.tile([P, 3 * n_sub], F32)
        nc.sync.dma_start(wp_t[:], wp[:])
        iota_t = const.tile([P, P], F32)
        nc.gpsimd.iota(iota_t[:], pattern=[[1, P]], base=0, channel_multiplier=0,
                       allow_small_or_imprecise_dtypes=True)

        for base_w in range(0, W, 2):
            pair = [w for w in (base_w, base_w + 1) if w < W]

            xsn_t, edg_t = {}, {}
            for j, w in enumerate(pair):
                sl_w = slice(w * SLOTS_WIN, (w + 1) * SLOTS_WIN)
                xsn_t[w] = io.tile([P, SLOTS_WIN], dt, tag=f"xsn{j}",
                                   name=f"xsn{j}")
                nc.sync.dma_start(xsn_t[w][:], xsn[:, sl_w])
                edg_t[w] = io.tile([P, SLOTS_WIN], dt, tag=f"edg{j}",
                                   name=f"edg{j}")
                nc.sync.dma_start(edg_t[w][:], edg[:, sl_w])

            # ---------------- edge MLP (feature-major), pair-interleaved ------
            ecur = {w: edg_t[w] for w in pair}
            for l in range(5):
                ps_l = {}
                for w in pair:
                    ps = ps_big.tile([P, SLOTS_WIN], F32, tag="psbig")
                    for t in range(2):
                        sl_t = slice(t * 512, (t + 1) * 512)
                        nc.tensor.matmul(out=ps[:, sl_t],
                                         lhsT=ew_t[:, l * P:(l + 1) * P],
                                         rhs=ecur[w][:, sl_t], start=True, stop=True)
                    ps_l[w] = ps
                for j, w in enumerate(pair):
                    enew = actp.tile([P, SLOTS_WIN], dt, tag=f"ebuf{l % 2}{j}")
                    if l < 4:
                        nc.scalar.activation(enew[:], ps_l[w][:], act_fn,
                                             bias=eb_t[:, l:l + 1], scale=1.0,
                                             alpha=0.01)
                    else:
                        # final linear layer (no activation): bias-add
                        nc.scalar.activation(enew[:], ps_l[w][:], AFT.Identity,
                                             bias=eb_t[:, l:l + 1])
                    ecur[w] = enew
            e_out = ecur

            # ---------------- gate/msg layer 1 (feature-major) ----------------
            hid = {}
            for mi in range(2):          # 0 = gate, 1 = msg
                for h in range(H):
                    for o in range(2):   # output 128-block
                        ps_l = {}
                        blk0 = (((mi * 3 + h) * 2 + 0) * 2 + o) * P
                        blk1 = (((mi * 3 + h) * 2 + 1) * 2 + o) * P
                        for w in pair:
                            ps = ps_big.tile([P, SLOTS_WIN], F32, tag="psbig",
                                             name="ps")
                            for t in range(2):
                                sl_t = slice(t * 512, (t + 1) * 512)
                                nc.tensor.matmul(out=ps[:, sl_t],
                                                 lhsT=w1_t[:, blk0:blk0 + P],
                                                 rhs=xsn_t[w][:, sl_t],
                                                 start=True, stop=False)
                                nc.tensor.matmul(out=ps[:, sl_t],
                                                 lhsT=w1_t[:, blk1:blk1 + P],
                                                 rhs=e_out[w][:, sl_t],
                                                 start=False, stop=True)
                            ps_l[w] = ps
                        bcol = (mi * 3 + h) * 2 + o
                        on_dve = False  # DVE leaky offload measured slower (v5)
                        for j, w in enumerate(pair):
                            hd = actp.tile([P, SLOTS_WIN], dt, tag=f"hid{mi}{h}{o}{j}")
                            if not on_dve:
                                nc.scalar.activation(hd[:], ps_l[w][:], act_fn,
                                                     bias=b1_t[:, bcol:bcol + 1],
                                                     scale=1.0, alpha=0.01)
                            else:
                                # leaky on DVE: y=x+b; out=max(y, 0.01*y)
                                yb = small.tile([P, SLOTS_WIN], dt, tag="dveA",
                                                name="yb")
                                nc.vector.tensor_scalar(out=yb[:], in0=ps_l[w][:],
                                                        scalar1=b1_t[:, bcol:bcol + 1],
                                                        scalar2=None, op0=ALU.add)
                                ys = small.tile([P, SLOTS_WIN], dt, tag="dveB",
                                                name="ys")
                                nc.vector.tensor_scalar(out=ys[:], in0=yb[:],
                                                        scalar1=0.01, scalar2=None,
                                                        op0=ALU.mult)
                                nc.vector.tensor_tensor(out=hd[:], in0=yb[:],
                                                        in1=ys[:], op=ALU.max)
                            hid[(w, mi, h, o)] = hd

            # ---------------- gate layer 2 (edge-major) + softmax numerators --
            # one PSUM tile + one Exp/mult/DMA for the whole pair
            npair = len(pair)
            ps_g = ps_gp.tile([P, npair * 3 * T_WIN], F32, tag="psg", name="ps_g")
            for jw, w in enumerate(pair):
                for s in range(T_WIN):
                    sl_s = slice(s * P, (s + 1) * P)
                    for h in range(H):
                        c = jw * 3 * T_WIN + s * 3 + h
                        nc.tensor.matmul(out=ps_g[:, c:c + 1],
                                         lhsT=hid[(w, 0, h, 0)][:, sl_s],
                                         rhs=g2w_t[:, h * 2:h * 2 + 1],
                                         start=True, stop=False)
                        nc.tensor.matmul(out=ps_g[:, c:c + 1],
                                         lhsT=hid[(w, 0, h, 1)][:, sl_s],
                                         rhs=g2w_t[:, h * 2 + 1:h * 2 + 2],
                                         start=False, stop=True)
            gexp = small.tile([P, npair * 3 * T_WIN], F32, tag="gexp", name="gexp")
            nc.scalar.activation(gexp[:], ps_g[:], AFT.Exp)
            gcol_pair = small.tile([P, npair * 3 * T_WIN], F32, tag="gcol",
                                   name="gcol_pair")
            w0 = pair[0]
            nc.vector.tensor_tensor(
                out=gcol_pair[:], in0=gexp[:],
                in1=wp_t[:, w0 * 3 * T_WIN:(w0 + npair) * 3 * T_WIN],
                op=ALU.mult)
            nc.sync.dma_start(gout[:, w0 * 3 * T_WIN:(w0 + npair) * 3 * T_WIN],
                              gcol_pair[:])
            gbase = {w: jw * 3 * T_WIN for jw, w in enumerate(pair)}

            # ---------------- msg layer 2 + segment matmuls ----------------
            for w in pair:
                gb = gbase[w]
                ps_w = ps_wp.tile([P, SEG_COLS], F32, tag="psw")
                for s in range(T_WIN):
                    sl_s = slice(s * P, (s + 1) * P)
                    ps_m = ps_mp.tile([P, H * ELEM], F32, tag="psm")
                    for h in range(H):
                        nc.tensor.matmul(out=ps_m[:, h * ELEM:(h + 1) * ELEM],
                                         lhsT=hid[(w, 1, h, 0)][:, sl_s],
                                         rhs=m2w_t[:, (h * 2) * ELEM:(h * 2 + 1) * ELEM],
                                         start=True, stop=False)
                        nc.tensor.matmul(out=ps_m[:, h * ELEM:(h + 1) * ELEM],
                                         lhsT=hid[(w, 1, h, 1)][:, sl_s],
                                         rhs=m2w_t[:, (h * 2 + 1) * ELEM:(h * 2 + 2) * ELEM],
                                         start=False, stop=True)
                    rhs_seg = small.tile([P, SEG_COLS], F32, tag="rhs")
                    for h in range(H):
                        c = gb + s * 3 + h
                        nc.vector.tensor_scalar(out=rhs_seg[:, h * ELEM:(h + 1) * ELEM],
                                                in0=ps_m[:, h * ELEM:(h + 1) * ELEM],
                                                scalar1=gcol_pair[:, c:c + 1],
                                                scalar2=None, op0=ALU.mult)
                    nc.vector.tensor_copy(rhs_seg[:, H * ELEM:H * ELEM + 3],
                                          gcol_pair[:, gb + s * 3:gb + s * 3 + 3])
                    onehot = small.tile([P, P], F32, tag="onehot")
                    q = w * T_WIN + s
                    nc.vector.tensor_tensor(out=onehot[:],
                                            in0=idxr_t[:, q:q + 1].to_broadcast([P, P]),
                                            in1=iota_t[:], op=ALU.is_equal)
                    nc.tensor.matmul(out=ps_w[:], lhsT=onehot[:], rhs=rhs_seg[:],
                                     start=(s == 0), stop=(s == T_WIN - 1))
                seg_sb = small.tile([P, SEG_COLS], F32, tag="segsb")
                nc.vector.tensor_copy(seg_sb[:], ps_w[:])
                nc.sync.dma_start(sout[:, w * SEG_COLS:(w + 1) * SEG_COLS], seg_sb[:])

    nc.compile()
    return nc


# --------------------------------------------------------------------------
# host-side packing
# --------------------------------------------------------------------------

def _pack(inputs, n_cores=N_CORES, np_dt=NP_DT):
    """Sort edges, pack windows, build per-core device arrays."""
    elem_weights = np.asarray(inputs["elem_weights"], np.float32)
    elem_in_fea = np.asarray(inputs["elem_in_fea"], np.float32)
    edge_fea = np.asarray(inputs["edge_fea"], np.float32)
    self_idx = np.asarray(inputs["self_fea_idx"]).astype(np.int64)
    nbr_idx = np.asarray(inputs["nbr_fea_idx"]).astype(np.int64)
    pow_p = np.asarray(inputs["pow_p"], np.float32)
    gate_b2 = np.asarray(inputs["gate_b2"], np.float32).reshape(-1)

    M = self_idx.shape[0]
    perm = np.argsort(self_idx, kind="stable")
    idx_s = self_idx[perm]
    nbr_s = nbr_idx[perm]

    # greedy window packing: <=1024 edges, node span < 128
    starts, ends, bases = [], [], []
    start = 0
    while start < M:
        base = int(idx_s[start])
        j_lim = int(np.searchsorted(idx_s, base + P, side="left"))
        end = min(start + SLOTS_WIN, j_lim)
        starts.append(start)
        ends.append(end)
        bases.append(base)
        start = end
    n_win = len(starts)
    W = -(-n_win // n_cores)          # windows per core
    n_win_pad = W * n_cores
    starts += [M] * (n_win_pad - n_win)
    ends += [M] * (n_win_pad - n_win)
    bases += [0] * (n_win_pad - n_win)
    starts = np.asarray(starts)
    ends = np.asarray(ends)
    bases = np.asarray(bases)

    n_sub = W * T_WIN
    n_slots = n_sub * P

    # slot -> sorted-edge position (-1 = pad), global over all cores
    cnt = ends - starts
    slot_pos = np.full((n_win_pad, SLOTS_WIN), -1, np.int64)
    arange_sw = np.arange(SLOTS_WIN)
    mask = arange_sw[None, :] < cnt[:, None]
    slot_pos[mask] = (starts[:, None] + arange_sw[None, :])[mask]

    # per-slot features
    flat_pos = slot_pos.reshape(-1)
    valid = flat_pos >= 0
    pos_v = flat_pos[valid]

    xsn_all = np.zeros((n_win_pad * SLOTS_WIN, P), np.float32)
    xsn_all[valid, :ELEM] = elem_in_fea[idx_s[pos_v]]
    xsn_all[valid, ELEM:] = elem_in_fea[nbr_s[pos_v]]
    edge_all = np.zeros((n_win_pad * SLOTS_WIN, EDGE), np.float32)
    edge_all[valid] = edge_fea[perm[pos_v]]

    idxr_all = np.full((n_win_pad * SLOTS_WIN,), -1.0, np.float32)
    idxr_all[valid] = (idx_s[pos_v] - np.repeat(bases, SLOTS_WIN)[valid]).astype(np.float32)

    wp_all = np.zeros((n_win_pad * SLOTS_WIN, H), np.float32)
    lnw = np.log(elem_weights[nbr_s[pos_v], 0])
    wp_all[valid] = np.exp(lnw[:, None] * pow_p[None, :] + gate_b2[None, :])

    # weights (shared across cores)
    ew = np.concatenate([np.asarray(inputs[f"edge_W{i}"], np.float32) for i in range(4)]
                        + [np.asarray(inputs["edge_Wo"], np.float32)], axis=1)
    eb = np.stack([np.asarray(inputs[f"edge_b{i}"], np.float32) for i in range(4)]
                  + [np.asarray(inputs["edge_bo"], np.float32)], axis=1)  # (128,5)
    gW1 = np.asarray(inputs["gate_W1"], np.float32)
    mW1 = np.asarray(inputs["msg_W1"], np.float32)
    gb1 = np.asarray(inputs["gate_b1"], np.float32)
    mb1 = np.asarray(inputs["msg_b1"], np.float32)
    w1 = np.zeros((P, 24 * P), np.float32)
    b1 = np.zeros((P, 12), np.float32)
    for mi, (W1m, b1m) in enumerate(((gW1, gb1), (mW1, mb1))):
        for h in range(H):
            for i in range(2):
                for o in range(2):
                    blk = (((mi * 3 + h) * 2 + i) * 2 + o) * P
                    w1[:, blk:blk + P] = W1m[h, i * P:(i + 1) * P, o * P:(o + 1) * P]
            for o in range(2):
                b1[:, (mi * 3 + h) * 2 + o] = b1m[h, o * P:(o + 1) * P]
    gW2 = np.asarray(inputs["gate_W2"], np.float32)
    mW2 = np.asarray(inputs["msg_W2"], np.float32)
    g2w = np.zeros((P, 6), np.float32)
    m2w = np.zeros((P, 6 * ELEM), np.float32)
    for h in range(H):
        for i in range(2):
            g2w[:, h * 2 + i] = gW2[h, i * P:(i + 1) * P, 0]
            m2w[:, (h * 2 + i) * ELEM:(h * 2 + i + 1) * ELEM] = mW2[h, i * P:(i + 1) * P]

    shared = dict(
        ew=np.ascontiguousarray(ew).astype(np_dt),
        eb=np.ascontiguousarray(eb),
        w1=np.ascontiguousarray(w1).astype(np_dt),
        b1=np.ascontiguousarray(b1),
        g2w=np.ascontiguousarray(g2w).astype(np_dt),
        m2w=np.ascontiguousarray(m2w).astype(np_dt),
    )

    in_maps = []
    for c in range(n_cores):
        sl = slice(c * W * SLOTS_WIN, (c + 1) * W * SLOTS_WIN)
        xsn_c = np.ascontiguousarray(xsn_all[sl].T).astype(np_dt)          # (128, n_slots)
        edg_c = np.ascontiguousarray(edge_all[sl].T).astype(np_dt)
        idxr_c = np.ascontiguousarray(
            idxr_all[sl].reshape(n_sub, P).T)                               # (128, n_sub)
        wp_c = np.ascontiguousarray(
            wp_all[sl].reshape(n_sub, P, H).transpose(1, 0, 2).reshape(P, 3 * n_sub))
        m = dict(xsn=xsn_c, edg=edg_c, idxr=idxr_c, wp=wp_c)
        m.update(shared)
        in_maps.append(m)

    meta = dict(perm=perm, idx_s=idx_s, slot_pos=slot_pos, bases=bases,
                n_win=n_win, n_win_pad=n_win_pad, W=W, n_sub=n_sub,
                n_slots=n_slots, M=M)
    return in_maps, meta


def _postprocess(results, meta, inputs):
    elem_in_fea = np.asarray(inputs["elem_in_fea"], np.float32)
    msg_b2 = np.asarray(inputs["msg_b2"], np.float32)        # (H, ELEM)
    N = elem_in_fea.shape[0]
    M = meta["M"]
    W = meta["W"]
    n_cores = len(results)

    # gather per-window segment partials
    seg = np.concatenate(
        [results[c]["sout"].reshape(P, W, SEG_COLS).transpose(1, 0, 2)
         for c in range(n_cores)], axis=0)                    # (n_win_pad, 128, 195)
    bases = meta["bases"]
    N_acc = int(bases.max()) + P
    acc = np.zeros((max(N_acc, N), SEG_COLS), np.float32)
    rows = (bases[:, None] + np.arange(P)[None, :]).reshape(-1)
    np.add.at(acc, rows, seg.reshape(-1, SEG_COLS))

    msum = acc[:N, :H * ELEM].reshape(N, H, ELEM)
    Z = acc[:N, H * ELEM:H * ELEM + H]                        # (N, H)

    outh = (msum + Z[:, :, None] * msg_b2[None, :, :]) / (Z[:, :, None] + 1e-10)
    out = outh.mean(axis=1) + elem_in_fea

    # gates
    g_slots = np.concatenate(
        [results[c]["gout"].reshape(P, W * T_WIN, H).transpose(1, 0, 2).reshape(-1, H)
         for c in range(n_cores)], axis=0)                    # (n_win_pad*1024, H)
    flat_pos = meta["slot_pos"].reshape(-1)
    valid = flat_pos >= 0
    g_sorted = np.empty((M, H), np.float32)
    g_sorted[flat_pos[valid]] = g_slots[valid]
    g_edge = np.empty((M, H), np.float32)
    g_edge[meta["perm"]] = g_sorted                           # back to original order
    idx = np.asarray(inputs["self_fea_idx"]).astype(np.int64)
    gates = g_edge / (Z[idx] + 1e-10)                         # (M, H)
    gates = np.ascontiguousarray(gates.T)[:, :, None]         # (H, M, 1)
    return out.astype(np.float32), gates.astype(np.float32)


# --------------------------------------------------------------------------
# entry point
# --------------------------------------------------------------------------

def kernel(**inputs):
    global LAST_RESULTS
    from concourse.bass_utils import run_bass_kernel_spmd

    in_maps, meta = _pack(inputs)
    key = (meta["W"], _DT_ENV)
    if key not in _PROGRAM_CACHE:
        _PROGRAM_CACHE[key] = build_program(meta["W"])
    nc = _PROGRAM_CACHE[key]

    trace = bool(os.environ.get("GNN_TRACE"))
    res = run_bass_kernel_spmd(nc, in_maps, list(range(N_CORES)), trace=trace)
    LAST_RESULTS = res
    return _postprocess(res.results, meta, inputs)


# revision 28
# speedup vs baseline: 1.0769x; 1.0769x over previous
"""Trainium2 Bass kernel for nn_MessageLayer (GNN message passing).

Strategy
--------
* Host: sort edges by self_fea_idx; pack sorted edges into "windows" of
  T_WIN*128 = 1024 edge slots whose node span is < 128 (so per-window
  segment reductions fit one 128-wide one-hot matmul).  Windows are
  distributed contiguously over the 8 cores; every core runs the exact
  same program (SPMD) on its own slice.
* Device (per core, per window):
    - edge MLP (5 layers, 128->128) feature-major, bf16 matmuls,
      fused bias+LeakyReLU on the scalar engine (PSUM->SBUF).
    - gate/msg layer-1 (256->256) feature-major.
    - layer-2 matmuls emit EDGE-major tiles directly by using the hidden
      activations as the stationary operand (no transposes).
    - g = exp(graw) * w^p  (softmax max-shift eliminated algebraically:
      a per-segment shift cancels in the ratio).
    - segment sums of [g*m | g] via one-hot matmuls accumulated in PSUM.
* Host post: accumulate per-window partials into (N, H, 64) sums and
  (N, H) softmax denominators, apply msg bias fold, normalization, head
  mean, residual; gates = g / (Z[self_idx] + 1e-10).

The kernel returns (out, gates) exactly like the reference:
  out: (N, 64) float32, gates: (H, M, 1) float32.
"""

import os
import sys
from contextlib import ExitStack

import numpy as np

for _p in ("/opt/trn_rl_repo",):
    if _p not in sys.path:
        sys.path.insert(0, _p)

import ml_dtypes  # noqa: E402
import concourse.bass as bass  # noqa: E402
import concourse.bacc as bacc  # noqa: E402
import concourse.mybir as mybir  # noqa: E402
import concourse.tile as tile  # noqa: E402

AFT = mybir.ActivationFunctionType
ALU = mybir.AluOpType
F32 = mybir.dt.float32
BF16 = mybir.dt.bfloat16

P = 128
H = 3
ELEM = 64
EDGE = 128
HID = 256
T_WIN = 8                 # 128-edge subtiles per window
SLOTS_WIN = T_WIN * P     # 1024 edge slots per window
SEG_COLS = H * ELEM + H   # 195: [gm heads | g heads]
N_CORES = 8

# matmul/activation dtype for the MLP path (fp32 for max accuracy, bf16 fast)
_DT_ENV = os.environ.get("GNN_KERNEL_DT", "bf16")
DT = BF16 if _DT_ENV == "bf16" else F32
NP_DT = ml_dtypes.bfloat16 if _DT_ENV == "bf16" else np.float32

LAST_RESULTS = None  # BassKernelResults of the most recent device run
_PROGRAM_CACHE = {}


# --------------------------------------------------------------------------
# device program
# --------------------------------------------------------------------------

def build_program(W, dt=DT, act_fn=AFT.Prelu):
    """Build the SPMD per-core Bass program for W windows per core."""
    n_sub = W * T_WIN
    n_slots = n_sub * P
    nc = bacc.Bacc("TRN2", target_bir_lowering=False, debug=False)

    xsn = nc.dram_tensor("xsn", [P, n_slots], dt, kind="ExternalInput")
    edg = nc.dram_tensor("edg", [P, n_slots], dt, kind="ExternalInput")
    idxr = nc.dram_tensor("idxr", [P, n_sub], F32, kind="ExternalInput")
    wp = nc.dram_tensor("wp", [P, 3 * n_sub], F32, kind="ExternalInput")
    ew = nc.dram_tensor("ew", [P, 5 * P], dt, kind="ExternalInput")
    eb = nc.dram_tensor("eb", [P, 5], F32, kind="ExternalInput")
    w1 = nc.dram_tensor("w1", [P, 24 * P], dt, kind="ExternalInput")
    b1 = nc.dram_tensor("b1", [P, 12], F32, kind="ExternalInput")
    g2w = nc.dram_tensor("g2w", [P, 6], dt, kind="ExternalInput")
    m2w = nc.dram_tensor("m2w", [P, 6 * ELEM], dt, kind="ExternalInput")
    gout = nc.dram_tensor("gout", [P, 3 * n_sub], F32, kind="ExternalOutput")
    sout = nc.dram_tensor("sout", [P, W * SEG_COLS], F32, kind="ExternalOutput")

    with ExitStack() as ctx:
        is16 = dt == BF16
        tc = ctx.enter_context(tile.TileContext(nc))
        const = ctx.enter_context(tc.tile_pool(name="const", bufs=1))
        io = ctx.enter_context(tc.tile_pool(name="io", bufs=3 if is16 else 2))
        actp = ctx.enter_context(tc.tile_pool(name="act", bufs=2 if is16 else 1))
        small = ctx.enter_context(tc.tile_pool(name="small", bufs=4))
        ps_big = ctx.enter_context(tc.tile_pool(name="ps_big", bufs=2, space="PSUM"))
        ps_gp = ctx.enter_context(tc.tile_pool(name="ps_g", bufs=1, space="PSUM"))
        ps_mp = ctx.enter_context(tc.tile_pool(name="ps_m", bufs=2, space="PSUM"))
        ps_wp = ctx.enter_context(tc.tile_pool(name="ps_w", bufs=1, space="PSUM"))

        ew_t = const.tile([P, 5 * P], dt)
        nc.sync.dma_start(ew_t[:], ew[:])
        eb_t = const.tile([P, 5], F32)
        nc.sync.dma_start(eb_t[:], eb[:])
        w1_t = const.tile([P, 24 * P], dt)
        nc.sync.dma_start(w1_t[:], w1[:])
        b1_t = const.tile([P, 12], F32)
        nc.sync.dma_start(b1_t[:], b1[:])
        g2w_t = const.tile([P, 6], dt)
        nc.sync.dma_start(g2w_t[:], g2w[:])
        m2w_t = const.tile([P, 6 * ELEM], dt)
        nc.sync.dma_start(m2w_t[:], m2w[:])
        idxr_t = const.tile([P, n_sub], F32)
        nc.sync.dma_start(idxr_t[:], idxr[:])
        wp_t = const.tile([P, 3 * n_sub], F32)
        nc.sync.dma_start(wp_t[:], wp[:])
        iota_t = const.tile([P, P], F32)
        nc.gpsimd.iota(iota_t[:], pattern=[[1, P]], base=0, channel_multiplier=0,
                       allow_small_or_imprecise_dtypes=True)

        for base_w in range(0, W, 2):
            pair = [w for w in (base_w, base_w + 1) if w < W]

            xsn_t, edg_t = {}, {}
            for j, w in enumerate(pair):
                sl_w = slice(w * SLOTS_WIN, (w + 1) * SLOTS_WIN)
                xsn_t[w] = io.tile([P, SLOTS_WIN], dt, tag=f"xsn{j}",
                                   name=f"xsn{j}")
                nc.sync.dma_start(xsn_t[w][:], xsn[:, sl_w])
                edg_t[w] = io.tile([P, SLOTS_WIN], dt, tag=f"edg{j}",
                                   name=f"edg{j}")
                nc.sync.dma_start(edg_t[w][:], edg[:, sl_w])

            # ---------------- edge MLP (feature-major), pair-interleaved ------
            ecur = {w: edg_t[w] for w in pair}
            for l in range(5):
                ps_l = {}
                for w in pair:
                    ps = ps_big.tile([P, SLOTS_WIN], F32, tag="psbig")
                    for t in range(2):
                        sl_t = slice(t * 512, (t + 1) * 512)
                        nc.tensor.matmul(out=ps[:, sl_t],
                                         lhsT=ew_t[:, l * P:(l + 1) * P],
                                         rhs=ecur[w][:, sl_t], start=True, stop=True)
                    ps_l[w] = ps
                for j, w in enumerate(pair):
                    enew = actp.tile([P, SLOTS_WIN], dt, tag=f"ebuf{l % 2}{j}")
                    if l < 4:
                        nc.scalar.activation(enew[:], ps_l[w][:], act_fn,
                                             bias=eb_t[:, l:l + 1], scale=1.0,
                                             alpha=0.01)
                    else:
                        # final linear layer (no activation): bias-add
                        nc.scalar.activation(enew[:], ps_l[w][:], AFT.Identity,
                                             bias=eb_t[:, l:l + 1])
                    ecur[w] = enew
            e_out = ecur

            # ---------------- gate/msg layer 1 (feature-major) ----------------
            hid = {}
            for mi in range(2):          # 0 = gate, 1 = msg
                for h in range(H):
                    for o in range(2):   # output 128-block
                        ps_l = {}
                        blk0 = (((mi * 3 + h) * 2 + 0) * 2 + o) * P
                        blk1 = (((mi * 3 + h) * 2 + 1) * 2 + o) * P
                        for w in pair:
                            ps = ps_big.tile([P, SLOTS_WIN], F32, tag="psbig",
                                             name="ps")
                            for t in range(2):
                                sl_t = slice(t * 512, (t + 1) * 512)
                                nc.tensor.matmul(out=ps[:, sl_t],
                                                 lhsT=w1_t[:, blk0:blk0 + P],
                                                 rhs=xsn_t[w][:, sl_t],
                                                 start=True, stop=False)
                                nc.tensor.matmul(out=ps[:, sl_t],
                                                 lhsT=w1_t[:, blk1:blk1 + P],
                                                 rhs=e_out[w][:, sl_t],
                                                 start=False, stop=True)
                            ps_l[w] = ps
                        bcol = (mi * 3 + h) * 2 + o
                        on_dve = False  # DVE leaky offload measured slower (v5)
                        for j, w in enumerate(pair):
                            hd = actp.tile([P, SLOTS_WIN], dt, tag=f"hid{mi}{h}{o}{j}")
                            if not on_dve:
                                nc.scalar.activation(hd[:], ps_l[w][:], act_fn,
                                                     bias=b1_t[:, bcol:bcol + 1],
                                                     scale=1.0, alpha=0.01)
                            else:
                                # leaky on DVE: y=x+b; out=max(y, 0.01*y)
                                yb = small.tile([P, SLOTS_WIN], dt, tag="dveA",
                                                name="yb")
                                nc.vector.tensor_scalar(out=yb[:], in0=ps_l[w][:],
                                                        scalar1=b1_t[:, bcol:bcol + 1],
                                                        scalar2=None, op0=ALU.add)
                                ys = small.tile([P, SLOTS_WIN], dt, tag="dveB",
                                                name="ys")
                                nc.vector.tensor_scalar(out=ys[:], in0=yb[:],
                                                        scalar1=0.01, scalar2=None,
                                                        op0=ALU.mult)
                                nc.vector.tensor_tensor(out=hd[:], in0=yb[:],
                                                        in1=ys[:], op=ALU.max)
                            hid[(w, mi, h, o)] = hd

            # ---------------- gate layer 2 (edge-major) + softmax numerators --
            gcols = {}
            for w in pair:
                ps_g = ps_gp.tile([P, 3 * T_WIN], F32, tag="psg")
                for s in range(T_WIN):
                    sl_s = slice(s * P, (s + 1) * P)
                    for h in range(H):
                        c = s * 3 + h
                        nc.tensor.matmul(out=ps_g[:, c:c + 1],
                                         lhsT=hid[(w, 0, h, 0)][:, sl_s],
                                         rhs=g2w_t[:, h * 2:h * 2 + 1],
                                         start=True, stop=False)
                        nc.tensor.matmul(out=ps_g[:, c:c + 1],
                                         lhsT=hid[(w, 0, h, 1)][:, sl_s],
                                         rhs=g2w_t[:, h * 2 + 1:h * 2 + 2],
                                         start=False, stop=True)
                gexp = small.tile([P, 3 * T_WIN], F32, tag="gexp")
                nc.scalar.activation(gexp[:], ps_g[:], AFT.Exp)
                gcol = small.tile([P, 3 * T_WIN], F32, tag="gcol")
                nc.vector.tensor_tensor(out=gcol[:], in0=gexp[:],
                                        in1=wp_t[:, w * 3 * T_WIN:(w + 1) * 3 * T_WIN],
                                        op=ALU.mult)
                nc.sync.dma_start(gout[:, w * 3 * T_WIN:(w + 1) * 3 * T_WIN],
                                  gcol[:])
                gcols[w] = gcol

            # ---------------- msg layer 2 + segment matmuls ----------------
            for w in pair:
                gcol = gcols[w]
                ps_w = ps_wp.tile([P, SEG_COLS], F32, tag="psw")
                for s in range(T_WIN):
                    sl_s = slice(s * P, (s + 1) * P)
                    ps_m = ps_mp.tile([P, H * ELEM], F32, tag="psm")
                    for h in range(H):
                        nc.tensor.matmul(out=ps_m[:, h * ELEM:(h + 1) * ELEM],
                                         lhsT=hid[(w, 1, h, 0)][:, sl_s],
                                         rhs=m2w_t[:, (h * 2) * ELEM:(h * 2 + 1) * ELEM],
                                         start=True, stop=False)
                        nc.tensor.matmul(out=ps_m[:, h * ELEM:(h + 1) * ELEM],
                                         lhsT=hid[(w, 1, h, 1)][:, sl_s],
                                         rhs=m2w_t[:, (h * 2 + 1) * ELEM:(h * 2 + 2) * ELEM],
                                         start=False, stop=True)
                    rhs_seg = small.tile([P, SEG_COLS], dt, tag="rhs")
                    for h in range(H):
                        nc.vector.tensor_scalar(out=rhs_seg[:, h * ELEM:(h + 1) * ELEM],
                                                in0=ps_m[:, h * ELEM:(h + 1) * ELEM],
                                                scalar1=gcol[:, s * 3 + h:s * 3 + h + 1],
                                                scalar2=None, op0=ALU.mult)
                    nc.vector.tensor_copy(rhs_seg[:, H * ELEM:H * ELEM + 3],
                                          gcol[:, s * 3:s * 3 + 3])
                    onehot = small.tile([P, P], dt, tag="onehot")
                    q = w * T_WIN + s
                    nc.vector.tensor_tensor(out=onehot[:],
                                            in0=idxr_t[:, q:q + 1].to_broadcast([P, P]),
                                            in1=iota_t[:], op=ALU.is_equal)
                    nc.tensor.matmul(out=ps_w[:], lhsT=onehot[:], rhs=rhs_seg[:],
                                     start=(s == 0), stop=(s == T_WIN - 1))
                seg_sb = small.tile([P, SEG_COLS], F32, tag="segsb")
                nc.vector.tensor_copy(seg_sb[:], ps_w[:])
                nc.sync.dma_start(sout[:, w * SEG_COLS:(w + 1) * SEG_COLS], seg_sb[:])

    nc.compile()
    return nc


# --------------------------------------------------------------------------
# host-side packing
# --------------------------------------------------------------------------

def _pack(inputs, n_cores=N_CORES, np_dt=NP_DT):
    """Sort edges, pack windows, build per-core device arrays."""
    elem_weights = np.asarray(inputs["elem_weights"], np.float32)
    elem_in_fea = np.asarray(inputs["elem_in_fea"], np.float32)
    edge_fea = np.asarray(inputs["edge_fea"], np.float32)
    self_idx = np.asarray(inputs["self_fea_idx"]).astype(np.int64)
    nbr_idx = np.asarray(inputs["nbr_fea_idx"]).astype(np.int64)
    pow_p = np.asarray(inputs["pow_p"], np.float32)
    gate_b2 = np.asarray(inputs["gate_b2"], np.float32).reshape(-1)

    M = self_idx.shape[0]
    perm = np.argsort(self_idx, kind="stable")
    idx_s = self_idx[perm]
    nbr_s = nbr_idx[perm]

    # greedy window packing: <=1024 edges, node span < 128
    starts, ends, bases = [], [], []
    start = 0
    while start < M:
        base = int(idx_s[start])
        j_lim = int(np.searchsorted(idx_s, base + P, side="left"))
        end = min(start + SLOTS_WIN, j_lim)
        starts.append(start)
        ends.append(end)
        bases.append(base)
        start = end
    n_win = len(starts)
    W = -(-n_win // n_cores)          # windows per core
    n_win_pad = W * n_cores
    starts += [M] * (n_win_pad - n_win)
    ends += [M] * (n_win_pad - n_win)
    bases += [0] * (n_win_pad - n_win)
    starts = np.asarray(starts)
    ends = np.asarray(ends)
    bases = np.asarray(bases)

    n_sub = W * T_WIN
    n_slots = n_sub * P

    # slot -> sorted-edge position (-1 = pad), global over all cores
    cnt = ends - starts
    slot_pos = np.full((n_win_pad, SLOTS_WIN), -1, np.int64)
    arange_sw = np.arange(SLOTS_WIN)
    mask = arange_sw[None, :] < cnt[:, None]
    slot_pos[mask] = (starts[:, None] + arange_sw[None, :])[mask]

    # per-slot features
    flat_pos = slot_pos.reshape(-1)
    valid = flat_pos >= 0
    pos_v = flat_pos[valid]

    xsn_all = np.zeros((n_win_pad * SLOTS_WIN, P), np.float32)
    xsn_all[valid, :ELEM] = elem_in_fea[idx_s[pos_v]]
    xsn_all[valid, ELEM:] = elem_in_fea[nbr_s[pos_v]]
    edge_all = np.zeros((n_win_pad * SLOTS_WIN, EDGE), np.float32)
    edge_all[valid] = edge_fea[perm[pos_v]]

    idxr_all = np.full((n_win_pad * SLOTS_WIN,), -1.0, np.float32)
    idxr_all[valid] = (idx_s[pos_v] - np.repeat(bases, SLOTS_WIN)[valid]).astype(np.float32)

    wp_all = np.zeros((n_win_pad * SLOTS_WIN, H), np.float32)
    lnw = np.log(elem_weights[nbr_s[pos_v], 0])
    wp_all[valid] = np.exp(lnw[:, None] * pow_p[None, :] + gate_b2[None, :])

    # weights (shared across cores)
    ew = np.concatenate([np.asarray(inputs[f"edge_W{i}"], np.float32) for i in range(4)]
                        + [np.asarray(inputs["edge_Wo"], np.float32)], axis=1)
    eb = np.stack([np.asarray(inputs[f"edge_b{i}"], np.float32) for i in range(4)]
                  + [np.asarray(inputs["edge_bo"], np.float32)], axis=1)  # (128,5)
    gW1 = np.asarray(inputs["gate_W1"], np.float32)
    mW1 = np.asarray(inputs["msg_W1"], np.float32)
    gb1 = np.asarray(inputs["gate_b1"], np.float32)
    mb1 = np.asarray(inputs["msg_b1"], np.float32)
    w1 = np.zeros((P, 24 * P), np.float32)
    b1 = np.zeros((P, 12), np.float32)
    for mi, (W1m, b1m) in enumerate(((gW1, gb1), (mW1, mb1))):
        for h in range(H):
            for i in range(2):
                for o in range(2):
                    blk = (((mi * 3 + h) * 2 + i) * 2 + o) * P
                    w1[:, blk:blk + P] = W1m[h, i * P:(i + 1) * P, o * P:(o + 1) * P]
            for o in range(2):
                b1[:, (mi * 3 + h) * 2 + o] = b1m[h, o * P:(o + 1) * P]
    gW2 = np.asarray(inputs["gate_W2"], np.float32)
    mW2 = np.asarray(inputs["msg_W2"], np.float32)
    g2w = np.zeros((P, 6), np.float32)
    m2w = np.zeros((P, 6 * ELEM), np.float32)
    for h in range(H):
        for i in range(2):
            g2w[:, h * 2 + i] = gW2[h, i * P:(i + 1) * P, 0]
            m2w[:, (h * 2 + i) * ELEM:(h * 2 + i + 1) * ELEM] = mW2[h, i * P:(i + 1) * P]

    shared = dict(
        ew=np.ascontiguousarray(ew).astype(np_dt),
        eb=np.ascontiguousarray(eb),
        w1=np.ascontiguousarray(w1).astype(np_dt),
        b1=np.ascontiguousarray(b1),
        g2w=np.ascontiguousarray(g2w).astype(np_dt),
        m2w=np.ascontiguousarray(m2w).astype(np_dt),
    )

    in_maps = []
    for c in range(n_cores):
        sl = slice(c * W * SLOTS_WIN, (c + 1) * W * SLOTS_WIN)
        xsn_c = np.ascontiguousarray(xsn_all[sl].T).astype(np_dt)          # (128, n_slots)
        edg_c = np.ascontiguousarray(edge_all[sl].T).astype(np_dt)
        idxr_c = np.ascontiguousarray(
            idxr_all[sl].reshape(n_sub, P).T)                               # (128, n_sub)
        wp_c = np.ascontiguousarray(
            wp_all[sl].reshape(n_sub, P, H).transpose(1, 0, 2).reshape(P, 3 * n_sub))
        m = dict(xsn=xsn_c, edg=edg_c, idxr=idxr_c, wp=wp_c)
        m.update(shared)
        in_maps.append(m)

    meta = dict(perm=perm, idx_s=idx_s, slot_pos=slot_pos, bases=bases,
                n_win=n_win, n_win_pad=n_win_pad, W=W, n_sub=n_sub,
                n_slots=n_slots, M=M)
    return in_maps, meta


def _postprocess(results, meta, inputs):
    elem_in_fea = np.asarray(inputs["elem_in_fea"], np.float32)
    msg_b2 = np.asarray(inputs["msg_b2"], np.float32)        # (H, ELEM)
    N = elem_in_fea.shape[0]
    M = meta["M"]
    W = meta["W"]
    n_cores = len(results)

    # gather per-window segment partials
    seg = np.concatenate(
        [results[c]["sout"].reshape(P, W, SEG_COLS).transpose(1, 0, 2)
         for c in range(n_cores)], axis=0)                    # (n_win_pad, 128, 195)
    bases = meta["bases"]
    N_acc = int(bases.max()) + P
    acc = np.zeros((max(N_acc, N), SEG_COLS), np.float32)
    rows = (bases[:, None] + np.arange(P)[None, :]).reshape(-1)
    np.add.at(acc, rows, seg.reshape(-1, SEG_COLS))

    msum = acc[:N, :H * ELEM].reshape(N, H, ELEM)
    Z = acc[:N, H * ELEM:H * ELEM + H]                        # (N, H)

    outh = (msum + Z[:, :, None] * msg_b2[None, :, :]) / (Z[:, :, None] + 1e-10)
    out = outh.mean(axis=1) + elem_in_fea

    # gates
    g_slots = np.concatenate(
        [results[c]["gout"].reshape(P, W * T_WIN, H).transpose(1, 0, 2).reshape(-1, H)
         for c in range(n_cores)], axis=0)                    # (n_win_pad*1024, H)
    flat_pos = meta["slot_pos"].reshape(-1)
    valid = flat_pos >= 0
    g_sorted = np.empty((M, H), np.float32)
    g_sorted[flat_pos[valid]] = g_slots[valid]
    g_edge = np.empty((M, H), np.float32)
    g_edge[meta["perm"]] = g_sorted                           # back to original order
    idx = np.asarray(inputs["self_fea_idx"]).astype(np.int64)
    gates = g_edge / (Z[idx] + 1e-10)                         # (M, H)
    gates = np.ascontiguousarray(gates.T)[:, :, None]         # (H, M, 1)
    return out.astype(np.float32), gates.astype(np.float32)


# --------------------------------------------------------------------------
# entry point
# --------------------------------------------------------------------------

def kernel(**inputs):
    global LAST_RESULTS
    from concourse.bass_utils import run_bass_kernel_spmd

    in_maps, meta = _pack(inputs)
    key = (meta["W"], _DT_ENV)
    if key not in _PROGRAM_CACHE:
        _PROGRAM_CACHE[key] = build_program(meta["W"])
    nc = _PROGRAM_CACHE[key]

    trace = bool(os.environ.get("GNN_TRACE"))
    res = run_bass_kernel_spmd(nc, in_maps, list(range(N_CORES)), trace=trace)
    LAST_RESULTS = res
    return _postprocess(res.results, meta, inputs)
